# revision 36
# baseline (speedup 1.0000x reference)
"""Trainium2 Bass kernel for nn_CrossAttention (B=8, Lq=Lkv=1024, EQ=1024,
EKV=768, H=8, HD=128).

Sharding: pure data-parallel over batch — core b computes batch element b
(all 8 heads), no collectives. Each core:

  1. PE-transposes query/key/value so the embedding dim sits on partitions,
     then projects: QT[h]=[HD,Lq], KT[h]=[HD,Lkv], Vnat=[Lkv, H*HD].
  2. Per head, two score passes on PE (both needed because softmax+attn
     output want Lq on partitions while attn@V wants Lkv on partitions;
     recomputing scores on PE is cheaper than transposing exp through
     PSUM). Pass B computes scores=[Lq,Lkv] in f32r; its ACT exp carries
     accum_out so the softmax row-sums s come free; attn = araw * (1/s)
     per-partition, DMA'd out at full fp32/f32r precision. Pass A
     (interleaved with B to keep ACT streaming) computes scoresT=
     [Lkv,Lq]; its exp lands in bf16 (feeds only the `out` path).
     O'T[h] = V[h].T-contract expT accumulates in PSUM and is evacuated
     with the 1/s normalization folded in as a tensor_tensor multiply
     against a partition-broadcast reciprocal row (1/s transposed on PE
     and bounced through DRAM to broadcast across partitions).
  3. Output projection: P' = OT_cat.T @ proj_wT (bf16) accumulated over
     heads on PE, bias folded in as a K=1 rank-1 matmul. proj_w.T is
     built by PE transposes spread across the early heads as filler.

Precision: the attention path (Q/K projections, both score passes) runs
float32r (~5e-4 scale-rel error on attn); the value/output path (V
projection, expT, O, out projection) runs bf16 (~5e-3 on out).
"""

import numpy as np

B, LQ, LKV = 8, 1024, 1024
EQ, EKV, H, HD = 1024, 768, 8, 128
P = 128
NEQ, NEKV, NLQ, NLKV = EQ // P, EKV // P, LQ // P, LKV // P
SCALE = 1.0 / float(np.sqrt(HD))
N_CORES = 8

_CACHE = {}


def _build_nc():
    import concourse.bacc as bacc
    import concourse.tile as tile
    from concourse import mybir

    f32 = mybir.dt.float32
    f32r = mybir.dt.float32r

    nc = bacc.Bacc(
        "TRN2",
        target_bir_lowering=False,
        debug=False,
        num_devices=N_CORES,
    )

    query = nc.dram_tensor("query_b", [LQ, EQ], f32, kind="ExternalInput").ap()
    key = nc.dram_tensor("key_b", [LKV, EKV], f32, kind="ExternalInput").ap()
    value = nc.dram_tensor("value_b", [LKV, EKV], f32, kind="ExternalInput").ap()
    w_q = nc.dram_tensor("w_q", [H, EQ, HD], f32, kind="ExternalInput").ap()
    w_k = nc.dram_tensor("w_k", [H, EKV, HD], f32, kind="ExternalInput").ap()
    w_v = nc.dram_tensor("w_v", [H, EKV, HD], f32, kind="ExternalInput").ap()
    proj_w = nc.dram_tensor("proj_w", [EQ, H * HD], f32, kind="ExternalInput").ap()
    proj_b = nc.dram_tensor("proj_b", [1, EQ], f32, kind="ExternalInput").ap()
    out_b = nc.dram_tensor("out_b", [LQ, EQ], f32, kind="ExternalOutput").ap()
    attn_b = nc.dram_tensor("attn_b", [H, LQ, LKV], f32, kind="ExternalOutput").ap()

    with tile.TileContext(nc, pool_alloc_mode="queue") as tc:
        _emit(tc, query, key, value, w_q, w_k, w_v, proj_w, proj_b, out_b, attn_b)
    nc.compile()
    return nc


def _emit(tc, query, key, value, w_q, w_k, w_v, proj_w, proj_b, out_b, attn_b):
    from contextlib import ExitStack

    import concourse.bass as bass
    from concourse import mybir
    from concourse.masks import make_identity

    nc = tc.nc
    f32 = mybir.dt.float32
    f32r = mybir.dt.float32r
    bf16 = mybir.dt.bfloat16
    fp16 = mybir.dt.float16
    AF = mybir.ActivationFunctionType

    with ExitStack() as ctx:
        const = ctx.enter_context(tc.tile_pool(name="const", bufs=1))
        ident = const.tile([P, P], f32)
        make_identity(nc, ident)
        ones_f32 = const.tile([P, P], f32)
        nc.vector.memset(ones_f32, 1.0)
        ones_row = const.tile([1, P], bf16)
        nc.vector.tensor_copy(ones_row, ones_f32[0:1, :])

        persist = ctx.enter_context(tc.tile_pool(name="persist", bufs=1))
        QT = persist.tile([P, H, LQ], fp16)  # [hd, h, lq]
        KT = persist.tile([P, H, LKV], fp16)  # [hd, h, lkv]
        Vn = persist.tile([P, NLKV, H * HD], bf16)  # [lkv%128, lkv_t, (h d)]
        # OT[:, h, lq] = (attn[h] @ V[h]).T, normalized  -- [HD, H, LQ]
        OT = persist.tile([P, H, LQ], bf16)

        # ---------------- Phase 1: transposes + projections ----------------
        # phase-2 SBUF pools open FIRST so their addresses don't overlap the
        # phase-1 transients (stack allocator) -- lets early heads' exps run
        # while the later projections are still in flight.
        expp = ctx.enter_context(tc.tile_pool(name="expp", bufs=2))
        attnp = ctx.enter_context(tc.tile_pool(name="attnp", bufs=8))
        rbcp = ctx.enter_context(tc.tile_pool(name="rbcp", bufs=1))
        srowp = ctx.enter_context(tc.tile_pool(name="srowp", bufs=1))
        scolp = ctx.enter_context(tc.tile_pool(name="scolp", bufs=2))
        pwp = ctx.enter_context(tc.tile_pool(name="pwp", bufs=1))
        with ExitStack() as p1:
            natp = p1.enter_context(tc.tile_pool(name="natp", bufs=1))
            xtp = p1.enter_context(tc.tile_pool(name="xtp", bufs=1))
            wp = p1.enter_context(tc.tile_pool(name="wp", bufs=2))
            pst = p1.enter_context(tc.tile_pool(name="pst", bufs=4, space="PSUM"))
            psq = p1.enter_context(tc.tile_pool(name="psq", bufs=2, space="PSUM"))

            def transpose_in(x, rows, cols, dt, tag="xT"):
                # x: [rows, cols] DRAM -> xT [P, cols//P, rows] SBUF
                nk = cols // P
                nt = rows // P
                xT = xtp.tile([P, nk, rows], dt, tag=tag, name="xT")
                for t0 in range(0, nt, 2):  # quarters: 2 row-tiles per load
                    nat = natp.tile([P, 2, cols], f32, tag="nat", name="nat")
                    nc.sync.dma_start(
                        out=nat,
                        in_=x[t0 * P : (t0 + 2) * P, :].rearrange(
                            "(t p) e -> p t e", p=P
                        ),
                    )
                    for k in range(nk):
                        ps = pst.tile([P, 2 * P], f32, tag="pst", name="pst")
                        for t in range(2):
                            nc.tensor.transpose(
                                ps[:, t * P : (t + 1) * P],
                                nat[:, t, k * P : (k + 1) * P],
                                ident,
                            )
                        nc.vector.tensor_copy(
                            xT[:, k, t0 * P : (t0 + 2) * P], ps
                        )
                return xT

            # -- query/key transposed, then QT/KT interleaved per head so
            # head 0's attention dependencies complete as early as possible
            queryT = transpose_in(query, LQ, EQ, fp16)
            keyT = transpose_in(key, LKV, EKV, fp16, tag="xTk")
            for h in range(H):
                wq_t = wp.tile([P, NEQ, HD], fp16, tag="w", name="wq_t")
                nc.gpsimd.dma_start(
                    out=wq_t, in_=w_q[h].rearrange("(k p) d -> p k d", p=P)
                )
                for c in range(2):
                    pq = psq.tile([P, 512], f32, tag="psq", name="pq")
                    for k in range(NEQ):
                        nc.tensor.matmul(
                            pq,
                            wq_t[:, k, :],
                            queryT[:, k, c * 512 : (c + 1) * 512],
                            start=(k == 0),
                            stop=(k == NEQ - 1),
                        )
                    nc.vector.tensor_copy(QT[:, h, c * 512 : (c + 1) * 512], pq)
                wk_t = wp.tile([P, NEKV, HD], fp16, tag="w", name="wk_t")
                nc.gpsimd.dma_start(
                    out=wk_t, in_=w_k[h].rearrange("(k p) d -> p k d", p=P)
                )
                for c in range(2):
                    pk = psq.tile([P, 512], f32, tag="psq", name="pk")
                    for k in range(NEKV):
                        nc.tensor.matmul(
                            pk,
                            wk_t[:, k, :],
                            keyT[:, k, c * 512 : (c + 1) * 512],
                            start=(k == 0),
                            stop=(k == NEKV - 1),
                        )
                    nc.vector.tensor_copy(KT[:, h, c * 512 : (c + 1) * 512], pk)

            # -- value -> V natural [lkv, (h d)]  (reuses queryT's slot)
            valueT = transpose_in(value, LKV, EKV, bf16)
            for g in range(2):  # 4-head groups
                wv_g = wp.tile(
                    [P, NEKV, 4, HD], bf16, tag="wv", name="wv_g", bufs=1
                )
                for k in range(NEKV):
                    nc.gpsimd.dma_start(
                        out=wv_g[:, k],
                        in_=w_v[g * 4 : (g + 1) * 4, k * P : (k + 1) * P, :].rearrange(
                            "h p d -> p h d"
                        ),
                    )
                for lkv_t in range(NLKV):
                    pv = psq.tile([P, 512], f32, tag="psq", name="pv")
                    for k in range(NEKV):
                        nc.tensor.matmul(
                            pv,
                            valueT[:, k, lkv_t * P : (lkv_t + 1) * P],
                            wv_g[:, k].rearrange("p h d -> p (h d)"),
                            start=(k == 0),
                            stop=(k == NEKV - 1),
                        )
                    nc.vector.tensor_copy(
                        Vn[:, lkv_t, g * 512 : (g + 1) * 512], pv
                    )

        # ---------------- Phase 2: attention per head ----------------
        pwTp = ctx.enter_context(tc.tile_pool(name="pwTp", bufs=1))
        pwT = pwTp.tile([P, H, EQ], bf16)  # proj_w.T, built during phase 2

        with ExitStack() as p2:
            psA = p2.enter_context(tc.tile_pool(name="psA", bufs=1, space="PSUM"))
            psB = p2.enter_context(tc.tile_pool(name="psB", bufs=1, space="PSUM"))
            psO = p2.enter_context(tc.tile_pool(name="psO", bufs=1, space="PSUM"))
            psT = p2.enter_context(tc.tile_pool(name="psT", bufs=1, space="PSUM"))
            dramp = p2.enter_context(tc.tile_pool(name="dramp", bufs=2, space="DRAM"))

            for h in range(H):
                # build 1/4 of proj_w.T per head (h=2..5): PE filler work
                if 2 <= h < 6:
                    t0 = 2 * (h - 2)
                    pwn = pwp.tile([P, 2, H * HD], f32, tag="pwn", name="pwn")
                    nc.sync.dma_start(
                        out=pwn,
                        in_=proj_w[t0 * P : (t0 + 2) * P, :].rearrange(
                            "(t p) e -> p t e", p=P
                        ),
                    )
                    for k in range(H):
                        ps3 = psT.tile([P, 2 * P], f32, tag="pwt", name="ps3")
                        for t in range(2):
                            nc.tensor.transpose(
                                ps3[:, t * P : (t + 1) * P],
                                pwn[:, t, k * P : (k + 1) * P],
                                ident,
                            )
                        nc.vector.tensor_copy(
                            pwT[:, k, t0 * P : (t0 + 2) * P], ps3
                        )
                # ---- passes B+A interleaved: B computes scores natural
                # (attn output + accum_out row sums); A computes scoresT
                # whose exp (bf16) feeds the out path. Alternating B/A exps
                # keeps ACT streaming while single-buffered psums refill.
                s_all = scolp.tile([P, NLQ], f32, tag="s_all", name="s_all")
                araws = []
                expT = expp.tile([P, NLKV, LQ], bf16, tag="expT", name="expT")

                def emit_B(i):
                    pB = psB.tile([P, LKV], f32, tag="psB", name="pB")
                    for c in range(2):
                        nc.tensor.matmul(
                            pB[:, c * 512 : (c + 1) * 512],
                            QT[:, h, i * P : (i + 1) * P],
                            KT[:, h, c * 512 : (c + 1) * 512],
                            start=True,
                            stop=True,
                        )
                    araw = attnp.tile([P, LKV], f32, tag="attn", name="araw")
                    nc.scalar.activation(
                        araw, pB, AF.Exp, scale=SCALE,
                        accum_out=s_all[:, i : i + 1],
                    )
                    araws.append(araw)

                def emit_A(i):
                    pA = psA.tile([P, LQ], f32, tag="psA", name="pA")
                    for c in range(2):
                        nc.tensor.matmul(
                            pA[:, c * 512 : (c + 1) * 512],
                            KT[:, h, i * P : (i + 1) * P],
                            QT[:, h, c * 512 : (c + 1) * 512],
                            start=True,
                            stop=True,
                        )
                    nc.scalar.activation(
                        expT[:, i, :], pA, AF.Exp, scale=SCALE
                    )

                for i in range(NLQ):
                    emit_B(i)
                    emit_A(i)

                # ---- 1/s: per-partition for attn, broadcast row (via PE
                # transpose + DRAM bounce) for the out path
                r_all = scolp.tile([P, NLQ], f32, tag="r_all", name="r_all")
                nc.vector.reciprocal(r_all, s_all)
                pT = psT.tile([NLQ, P], f32, tag="psT", name="pT")
                nc.tensor.transpose(pT, r_all, ident)
                srow8 = srowp.tile([NLQ, P], f32, tag="srow8", name="srow8")
                nc.vector.tensor_copy(srow8, pT)
                srow_d = dramp.tile([NLQ, P], f32, tag="srow_d", name="srow_d")
                nc.sync.dma_start(out=srow_d, in_=srow8)
                rbc = rbcp.tile([P, LQ], f32, tag="rbc", name="rbc")
                nc.sync.dma_start(
                    out=rbc,
                    in_=bass.AP(
                        tensor=srow_d.tensor,
                        offset=srow_d.offset,
                        ap=[[0, P]] + [list(x) for x in srow_d.ap],
                    ),
                )
                for lq_t in range(NLQ):
                    nc.vector.tensor_scalar_mul(
                        araws[lq_t], araws[lq_t], r_all[:, lq_t : lq_t + 1]
                    )
                    nc.sync.dma_start(
                        out=attn_b[h, lq_t * P : (lq_t + 1) * P, :],
                        in_=araws[lq_t],
                    )

                # ---- O'T[h] = (exp @ V[h]).T, normalized on evacuation
                pO = psO.tile([P, LQ], f32, tag="psO", name="pO")
                for c in range(2):
                    for lkv_t in range(NLKV):
                        nc.tensor.matmul(
                            pO[:, c * 512 : (c + 1) * 512],
                            Vn[:, lkv_t, h * HD : (h + 1) * HD],
                            expT[:, lkv_t, c * 512 : (c + 1) * 512],
                            start=(lkv_t == 0),
                            stop=(lkv_t == NLKV - 1),
                        )
                for c in range(2):
                    nc.vector.tensor_mul(
                        OT[:, h, c * 512 : (c + 1) * 512],
                        pO[:, c * 512 : (c + 1) * 512],
                        rbc[:, c * 512 : (c + 1) * 512],
                    )

        # ---------------- Phase 3: output projection ----------------
        with ExitStack() as p3:
            outp = p3.enter_context(tc.tile_pool(name="outp", bufs=3))
            psP = p3.enter_context(tc.tile_pool(name="psP", bufs=3, space="PSUM"))

            bias_sb = outp.tile([1, EQ], bf16, bufs=1)
            nc.gpsimd.dma_start(out=bias_sb, in_=proj_b)

            for lq_t in range(NLQ):
                osb = outp.tile([P, EQ], f32, tag="osb", name="osb")
                for c in range(2):
                    pP = psP.tile([P, 512], f32, tag="psP", name="pP")
                    for hh in range(H):
                        nc.tensor.matmul(
                            pP,
                            OT[:, hh, lq_t * P : (lq_t + 1) * P],
                            pwT[:, hh, c * 512 : (c + 1) * 512],
                            start=(hh == 0),
                            stop=False,
                        )
                    nc.tensor.matmul(
                        pP,
                        ones_row,
                        bias_sb[:, c * 512 : (c + 1) * 512],
                        start=False,
                        stop=True,
                    )
                    nc.vector.tensor_copy(osb[:, c * 512 : (c + 1) * 512], pP)
                nc.sync.dma_start(
                    out=out_b[lq_t * P : (lq_t + 1) * P, :], in_=osb
                )


def _get_nc():
    if "nc" not in _CACHE:
        _CACHE["nc"] = _build_nc()
    return _CACHE["nc"]


def run(inputs, trace=False):
    from concourse.bass_utils import run_bass_kernel_spmd

    nc = _get_nc()
    q = np.ascontiguousarray(np.asarray(inputs["query"], dtype=np.float32))
    k = np.ascontiguousarray(np.asarray(inputs["key"], dtype=np.float32))
    v = np.ascontiguousarray(np.asarray(inputs["value"], dtype=np.float32))
    common = {
        "w_q": np.ascontiguousarray(np.asarray(inputs["w_q"], dtype=np.float32)),
        "w_k": np.ascontiguousarray(np.asarray(inputs["w_k"], dtype=np.float32)),
        "w_v": np.ascontiguousarray(np.asarray(inputs["w_v"], dtype=np.float32)),
        "proj_w": np.ascontiguousarray(
            np.asarray(inputs["proj_w"], dtype=np.float32)
        ),
        "proj_b": np.ascontiguousarray(
            np.asarray(inputs["proj_b"], dtype=np.float32)
        ).reshape(1, EQ),
    }
    in_maps = [
        {**common, "query_b": q[b], "key_b": k[b], "value_b": v[b]}
        for b in range(N_CORES)
    ]
    res = run_bass_kernel_spmd(
        nc, in_maps, core_ids=list(range(N_CORES)), trace=trace
    )
    out = np.stack([res.results[b]["out_b"] for b in range(N_CORES)])
    attn = np.stack(
        [res.results[b]["attn_b"] for b in range(N_CORES)], axis=1
    ).reshape(H * B, LQ, LKV)
    return (out, attn), res


def kernel(**inputs):
    (out, attn), _ = run(inputs, trace=False)
    return out, attn


# revision 38
# speedup vs baseline: 1.1868x; 1.1868x over previous
"""Trainium2 Bass kernel for nn_CrossAttention (B=8, Lq=Lkv=1024, EQ=1024,
EKV=768, H=8, HD=128).

Sharding: pure data-parallel over batch — core b computes batch element b
(all 8 heads), no collectives. Each core:

  1. PE-transposes query/key/value so the embedding dim sits on partitions,
     then projects: QT[h]=[HD,Lq], KT[h]=[HD,Lkv], Vnat=[Lkv, H*HD].
  2. Per head, two score passes on PE (both needed because softmax+attn
     output want Lq on partitions while attn@V wants Lkv on partitions;
     recomputing scores on PE is cheaper than transposing exp through
     PSUM). Pass B computes scores=[Lq,Lkv] in f32r; its ACT exp carries
     accum_out so the softmax row-sums s come free; attn = araw * (1/s)
     per-partition, DMA'd out at full fp32/f32r precision. Pass A
     (interleaved with B to keep ACT streaming) computes scoresT=
     [Lkv,Lq]; its exp lands in bf16 (feeds only the `out` path).
     O'T[h] = V[h].T-contract expT accumulates in PSUM and is evacuated
     with the 1/s normalization folded in as a tensor_tensor multiply
     against a partition-broadcast reciprocal row (1/s transposed on PE
     and bounced through DRAM to broadcast across partitions).
  3. Output projection: P' = OT_cat.T @ proj_wT (bf16) accumulated over
     heads on PE, bias folded in as a K=1 rank-1 matmul. proj_w.T is
     built by PE transposes spread across the early heads as filler.

Precision: the attention path (Q/K projections, both score passes) runs
fp16 (10-bit mantissa, tf32-class here; ~9e-4 scale-rel on attn); the
value/output path (V projection, expT, O, out projection) runs bf16
(~3.8e-3 on out, the binding error).
"""

import numpy as np

B, LQ, LKV = 8, 1024, 1024
EQ, EKV, H, HD = 1024, 768, 8, 128
P = 128
NEQ, NEKV, NLQ, NLKV = EQ // P, EKV // P, LQ // P, LKV // P
SCALE = 1.0 / float(np.sqrt(HD))
N_CORES = 8

_CACHE = {}


def _build_nc():
    import concourse.bacc as bacc
    import concourse.tile as tile
    from concourse import mybir

    f32 = mybir.dt.float32
    f32r = mybir.dt.float32r

    nc = bacc.Bacc(
        "TRN2",
        target_bir_lowering=False,
        debug=False,
        num_devices=N_CORES,
    )

    query = nc.dram_tensor("query_b", [LQ, EQ], f32, kind="ExternalInput").ap()
    key = nc.dram_tensor("key_b", [LKV, EKV], f32, kind="ExternalInput").ap()
    value = nc.dram_tensor("value_b", [LKV, EKV], f32, kind="ExternalInput").ap()
    w_q = nc.dram_tensor("w_q", [H, EQ, HD], f32, kind="ExternalInput").ap()
    w_k = nc.dram_tensor("w_k", [H, EKV, HD], f32, kind="ExternalInput").ap()
    w_v = nc.dram_tensor("w_v", [H, EKV, HD], f32, kind="ExternalInput").ap()
    proj_w = nc.dram_tensor("proj_w", [EQ, H * HD], f32, kind="ExternalInput").ap()
    proj_b = nc.dram_tensor("proj_b", [1, EQ], f32, kind="ExternalInput").ap()
    out_b = nc.dram_tensor("out_b", [LQ, EQ], f32, kind="ExternalOutput").ap()
    attn_b = nc.dram_tensor("attn_b", [H, LQ, LKV], f32, kind="ExternalOutput").ap()

    with tile.TileContext(nc, pool_alloc_mode="queue") as tc:
        _emit(tc, query, key, value, w_q, w_k, w_v, proj_w, proj_b, out_b, attn_b)
    nc.compile()
    return nc


def _emit(tc, query, key, value, w_q, w_k, w_v, proj_w, proj_b, out_b, attn_b):
    from contextlib import ExitStack

    import concourse.bass as bass
    from concourse import mybir
    from concourse.masks import make_identity

    nc = tc.nc
    f32 = mybir.dt.float32
    f32r = mybir.dt.float32r
    bf16 = mybir.dt.bfloat16
    fp16 = mybir.dt.float16
    AF = mybir.ActivationFunctionType

    with ExitStack() as ctx:
        const = ctx.enter_context(tc.tile_pool(name="const", bufs=1))
        ident = const.tile([P, P], f32)
        make_identity(nc, ident)
        ones_f32 = const.tile([P, P], f32)
        nc.vector.memset(ones_f32, 1.0)
        ones_row = const.tile([1, P], bf16)
        nc.vector.tensor_copy(ones_row, ones_f32[0:1, :])

        persist = ctx.enter_context(tc.tile_pool(name="persist", bufs=1))
        QT = persist.tile([P, H, LQ], fp16)  # [hd, h, lq]
        KT = persist.tile([P, H, LKV], fp16)  # [hd, h, lkv]
        Vn = persist.tile([P, NLKV, H * HD], bf16)  # [lkv%128, lkv_t, (h d)]
        # OT[:, h, lq] = (attn[h] @ V[h]).T, normalized  -- [HD, H, LQ]
        OT = persist.tile([P, H, LQ], bf16)

        # ---------------- Phase 1: transposes + projections ----------------
        with ExitStack() as p1:
            natp = p1.enter_context(tc.tile_pool(name="natp", bufs=3))
            xtp = p1.enter_context(tc.tile_pool(name="xtp", bufs=1))
            wp = p1.enter_context(tc.tile_pool(name="wp", bufs=2))
            pst = p1.enter_context(tc.tile_pool(name="pst", bufs=4, space="PSUM"))
            psq = p1.enter_context(tc.tile_pool(name="psq", bufs=2, space="PSUM"))

            def transpose_in(x, rows, cols, dt):
                # x: [rows, cols] DRAM -> xT [P, cols//P, rows] SBUF
                nk = cols // P
                nt = rows // P
                xT = xtp.tile([P, nk, rows], dt, tag="xT", name="xT")
                for t0 in range(0, nt, 2):  # quarters: 2 row-tiles per load
                    nat = natp.tile([P, 2, cols], f32, tag="nat", name="nat")
                    nc.sync.dma_start(
                        out=nat,
                        in_=x[t0 * P : (t0 + 2) * P, :].rearrange(
                            "(t p) e -> p t e", p=P
                        ),
                    )
                    for k in range(nk):
                        ps = pst.tile([P, 2 * P], f32, tag="pst", name="pst")
                        for t in range(2):
                            nc.tensor.transpose(
                                ps[:, t * P : (t + 1) * P],
                                nat[:, t, k * P : (k + 1) * P],
                                ident,
                            )
                        nc.vector.tensor_copy(
                            xT[:, k, t0 * P : (t0 + 2) * P], ps
                        )
                return xT

            # -- query -> QT
            queryT = transpose_in(query, LQ, EQ, fp16)
            for h in range(H):
                wq_t = wp.tile([P, NEQ, HD], fp16, tag="w", name="wq_t")
                nc.gpsimd.dma_start(
                    out=wq_t, in_=w_q[h].rearrange("(k p) d -> p k d", p=P)
                )
                for c in range(2):
                    pq = psq.tile([P, 512], f32, tag="psq", name="pq")
                    for k in range(NEQ):
                        nc.tensor.matmul(
                            pq,
                            wq_t[:, k, :],
                            queryT[:, k, c * 512 : (c + 1) * 512],
                            start=(k == 0),
                            stop=(k == NEQ - 1),
                        )
                    nc.vector.tensor_copy(QT[:, h, c * 512 : (c + 1) * 512], pq)

            # -- key -> KT
            keyT = transpose_in(key, LKV, EKV, fp16)
            for h in range(H):
                wk_t = wp.tile([P, NEKV, HD], fp16, tag="w", name="wk_t")
                nc.gpsimd.dma_start(
                    out=wk_t, in_=w_k[h].rearrange("(k p) d -> p k d", p=P)
                )
                for c in range(2):
                    pk = psq.tile([P, 512], f32, tag="psq", name="pk")
                    for k in range(NEKV):
                        nc.tensor.matmul(
                            pk,
                            wk_t[:, k, :],
                            keyT[:, k, c * 512 : (c + 1) * 512],
                            start=(k == 0),
                            stop=(k == NEKV - 1),
                        )
                    nc.vector.tensor_copy(KT[:, h, c * 512 : (c + 1) * 512], pk)

            # -- value -> V natural [lkv, (h d)]
            valueT = transpose_in(value, LKV, EKV, bf16)
            for g in range(2):  # 4-head groups
                wv_g = wp.tile(
                    [P, NEKV, 4, HD], bf16, tag="wv", name="wv_g", bufs=1
                )
                for k in range(NEKV):
                    nc.gpsimd.dma_start(
                        out=wv_g[:, k],
                        in_=w_v[g * 4 : (g + 1) * 4, k * P : (k + 1) * P, :].rearrange(
                            "h p d -> p h d"
                        ),
                    )
                for lkv_t in range(NLKV):
                    pv = psq.tile([P, 512], f32, tag="psq", name="pv")
                    for k in range(NEKV):
                        nc.tensor.matmul(
                            pv,
                            valueT[:, k, lkv_t * P : (lkv_t + 1) * P],
                            wv_g[:, k].rearrange("p h d -> p (h d)"),
                            start=(k == 0),
                            stop=(k == NEKV - 1),
                        )
                    nc.vector.tensor_copy(
                        Vn[:, lkv_t, g * 512 : (g + 1) * 512], pv
                    )

        # ---------------- Phase 2: attention per head ----------------
        pwTp = ctx.enter_context(tc.tile_pool(name="pwTp", bufs=1))
        pwT = pwTp.tile([P, H, EQ], bf16)  # proj_w.T, built during phase 2

        with ExitStack() as p2:
            expp = p2.enter_context(tc.tile_pool(name="expp", bufs=3))
            attnp = p2.enter_context(tc.tile_pool(name="attnp", bufs=10))
            rbcp = p2.enter_context(tc.tile_pool(name="rbcp", bufs=1))
            srowp = p2.enter_context(tc.tile_pool(name="srowp", bufs=1))
            scolp = p2.enter_context(tc.tile_pool(name="scolp", bufs=2))
            pwp = p2.enter_context(tc.tile_pool(name="pwp", bufs=1))
            psA = p2.enter_context(tc.tile_pool(name="psA", bufs=1, space="PSUM"))
            psB = p2.enter_context(tc.tile_pool(name="psB", bufs=1, space="PSUM"))
            psO = p2.enter_context(tc.tile_pool(name="psO", bufs=1, space="PSUM"))
            psT = p2.enter_context(tc.tile_pool(name="psT", bufs=1, space="PSUM"))
            dramp = p2.enter_context(tc.tile_pool(name="dramp", bufs=2, space="DRAM"))

            for h in range(H):
                # build 1/4 of proj_w.T per head (h=2..5): PE filler work
                if 2 <= h < 6:
                    t0 = 2 * (h - 2)
                    pwn = pwp.tile([P, 2, H * HD], f32, tag="pwn", name="pwn")
                    nc.sync.dma_start(
                        out=pwn,
                        in_=proj_w[t0 * P : (t0 + 2) * P, :].rearrange(
                            "(t p) e -> p t e", p=P
                        ),
                    )
                    for k in range(H):
                        ps3 = psT.tile([P, 2 * P], f32, tag="pwt", name="ps3")
                        for t in range(2):
                            nc.tensor.transpose(
                                ps3[:, t * P : (t + 1) * P],
                                pwn[:, t, k * P : (k + 1) * P],
                                ident,
                            )
                        nc.vector.tensor_copy(
                            pwT[:, k, t0 * P : (t0 + 2) * P], ps3
                        )
                # ---- passes B+A interleaved: B computes scores natural
                # (attn output + accum_out row sums); A computes scoresT
                # whose exp (bf16) feeds the out path. Alternating B/A exps
                # keeps ACT streaming while single-buffered psums refill.
                s_all = scolp.tile([P, NLQ], f32, tag="s_all", name="s_all")
                araws = []
                expT = expp.tile([P, NLKV, LQ], bf16, tag="expT", name="expT")

                def emit_B(i):
                    pB = psB.tile([P, LKV], f32, tag="psB", name="pB")
                    for c in range(2):
                        nc.tensor.matmul(
                            pB[:, c * 512 : (c + 1) * 512],
                            QT[:, h, i * P : (i + 1) * P],
                            KT[:, h, c * 512 : (c + 1) * 512],
                            start=True,
                            stop=True,
                        )
                    araw = attnp.tile([P, LKV], f32, tag="attn", name="araw")
                    nc.scalar.activation(
                        araw, pB, AF.Exp, scale=SCALE,
                        accum_out=s_all[:, i : i + 1],
                    )
                    araws.append(araw)

                def emit_A(i):
                    pA = psA.tile([P, LQ], f32, tag="psA", name="pA")
                    for c in range(2):
                        nc.tensor.matmul(
                            pA[:, c * 512 : (c + 1) * 512],
                            KT[:, h, i * P : (i + 1) * P],
                            QT[:, h, c * 512 : (c + 1) * 512],
                            start=True,
                            stop=True,
                        )
                    nc.scalar.activation(
                        expT[:, i, :], pA, AF.Exp, scale=SCALE
                    )

                for i in range(NLQ):
                    emit_B(i)
                    emit_A(i)

                # ---- 1/s: per-partition for attn, broadcast row (via PE
                # transpose + DRAM bounce) for the out path
                r_all = scolp.tile([P, NLQ], f32, tag="r_all", name="r_all")
                nc.vector.reciprocal(r_all, s_all)
                pT = psT.tile([NLQ, P], f32, tag="psT", name="pT")
                nc.tensor.transpose(pT, r_all, ident)
                srow8 = srowp.tile([NLQ, P], f32, tag="srow8", name="srow8")
                nc.vector.tensor_copy(srow8, pT)
                srow_d = dramp.tile([NLQ, P], f32, tag="srow_d", name="srow_d")
                nc.sync.dma_start(out=srow_d, in_=srow8)
                rbc = rbcp.tile([P, LQ], f32, tag="rbc", name="rbc")
                nc.sync.dma_start(
                    out=rbc,
                    in_=bass.AP(
                        tensor=srow_d.tensor,
                        offset=srow_d.offset,
                        ap=[[0, P]] + [list(x) for x in srow_d.ap],
                    ),
                )
                for lq_t in range(NLQ):
                    nc.vector.tensor_scalar_mul(
                        araws[lq_t], araws[lq_t], r_all[:, lq_t : lq_t + 1]
                    )
                    nc.sync.dma_start(
                        out=attn_b[h, lq_t * P : (lq_t + 1) * P, :],
                        in_=araws[lq_t],
                    )

                # ---- O'T[h] = (exp @ V[h]).T, normalized on evacuation
                pO = psO.tile([P, LQ], f32, tag="psO", name="pO")
                for c in range(2):
                    for lkv_t in range(NLKV):
                        nc.tensor.matmul(
                            pO[:, c * 512 : (c + 1) * 512],
                            Vn[:, lkv_t, h * HD : (h + 1) * HD],
                            expT[:, lkv_t, c * 512 : (c + 1) * 512],
                            start=(lkv_t == 0),
                            stop=(lkv_t == NLKV - 1),
                        )
                for c in range(2):
                    nc.vector.tensor_mul(
                        OT[:, h, c * 512 : (c + 1) * 512],
                        pO[:, c * 512 : (c + 1) * 512],
                        rbc[:, c * 512 : (c + 1) * 512],
                    )

        # ---------------- Phase 3: output projection ----------------
        with ExitStack() as p3:
            outp = p3.enter_context(tc.tile_pool(name="outp", bufs=3))
            psP = p3.enter_context(tc.tile_pool(name="psP", bufs=3, space="PSUM"))

            bias_sb = outp.tile([1, EQ], bf16, bufs=1)
            nc.gpsimd.dma_start(out=bias_sb, in_=proj_b)

            for lq_t in range(NLQ):
                osb = outp.tile([P, EQ], f32, tag="osb", name="osb")
                for c in range(2):
                    pP = psP.tile([P, 512], f32, tag="psP", name="pP")
                    for hh in range(H):
                        nc.tensor.matmul(
                            pP,
                            OT[:, hh, lq_t * P : (lq_t + 1) * P],
                            pwT[:, hh, c * 512 : (c + 1) * 512],
                            start=(hh == 0),
                            stop=False,
                        )
                    nc.tensor.matmul(
                        pP,
                        ones_row,
                        bias_sb[:, c * 512 : (c + 1) * 512],
                        start=False,
                        stop=True,
                    )
                    nc.vector.tensor_copy(osb[:, c * 512 : (c + 1) * 512], pP)
                nc.sync.dma_start(
                    out=out_b[lq_t * P : (lq_t + 1) * P, :], in_=osb
                )


def _get_nc():
    if "nc" not in _CACHE:
        _CACHE["nc"] = _build_nc()
    return _CACHE["nc"]


def run(inputs, trace=False):
    from concourse.bass_utils import run_bass_kernel_spmd

    nc = _get_nc()
    q = np.ascontiguousarray(np.asarray(inputs["query"], dtype=np.float32))
    k = np.ascontiguousarray(np.asarray(inputs["key"], dtype=np.float32))
    v = np.ascontiguousarray(np.asarray(inputs["value"], dtype=np.float32))
    common = {
        "w_q": np.ascontiguousarray(np.asarray(inputs["w_q"], dtype=np.float32)),
        "w_k": np.ascontiguousarray(np.asarray(inputs["w_k"], dtype=np.float32)),
        "w_v": np.ascontiguousarray(np.asarray(inputs["w_v"], dtype=np.float32)),
        "proj_w": np.ascontiguousarray(
            np.asarray(inputs["proj_w"], dtype=np.float32)
        ),
        "proj_b": np.ascontiguousarray(
            np.asarray(inputs["proj_b"], dtype=np.float32)
        ).reshape(1, EQ),
    }
    in_maps = [
        {**common, "query_b": q[b], "key_b": k[b], "value_b": v[b]}
        for b in range(N_CORES)
    ]
    res = run_bass_kernel_spmd(
        nc, in_maps, core_ids=list(range(N_CORES)), trace=trace
    )
    out = np.stack([res.results[b]["out_b"] for b in range(N_CORES)])
    attn = np.stack(
        [res.results[b]["attn_b"] for b in range(N_CORES)], axis=1
    ).reshape(H * B, LQ, LKV)
    return (out, attn), res


def kernel(**inputs):
    (out, attn), _ = run(inputs, trace=False)
    return out, attn
